# revision 18
# baseline (speedup 1.0000x reference)
"""Cross-graph attention (block-diagonal segment-local attention) on 8 trn2
cores — v3.

Math: out = atom_h + softmax_r(atom_h @ Wq^T Wk @ res_h^T / sqrt(128)) @ V,
segment-local per graph, V = res_h @ Wv^T.

Structure:
  - Wq folded into the key projection host-side: M = Wq^T Wk / sqrt(128),
    so scores stream atom_h^T directly (no Q matmul / copy on device).
  - bf16 matmul operands everywhere (1 cycle/row on PE at any free size),
    f32 PSUM. Verified ~6e-4 rel err vs the 2e-2 budget.
  - Graphs sorted by residue-chunk count then atom count into G slot
    classes of 8 (one graph per core per slot); per-slot (AG, nk) padding.
  - ctx^T = sum_k V_k^T @ ES_k with V chunks stationary, exp-scores moving.
  - softmax denominator: ES chunk partial-adds (gpsimd for early slots,
    DVE for the late ones) into ESsum; host does the final 128-row column
    sum. No masking bias anywhere: padded residues give exp(0)=1 which the
    host subtracts from the denominator (their V rows are zero, so ctx is
    untouched).
  - IO is packed to minimize DMA count (each DMA costs ~1.3us of issue +
    HWDGE time + 0.9us completion-semaphore): one input tensor [mT|wvT|resT]
    split in two DMAs, atomT in three, one combined [ctx|esum] output
    tensor, one DMA/slot (the last slot split in two to shorten the tail).
  - warm-up matmuls + a dummy exp during the DMA window hide the PE p-state
    ramp and the 1.3us activation-table load.
"""

import sys

if "/opt/trn_rl_repo" not in sys.path:
    sys.path.insert(0, "/opt/trn_rl_repo")

import ml_dtypes
import numpy as np

import concourse.bass as bass
import concourse.tile as tile
from concourse import bacc, mybir
from concourse.bass_utils import run_bass_kernel_spmd

N_CORES = 8
P = 128
DH = 128
NEG_BIAS = -30000.0
NWARM = 6
BF16 = ml_dtypes.bfloat16

_kernel_cache: dict = {}


def _build_kernel(spec):
    """spec: tuple of (AG_j, nk_j) per slot; one SPMD program for all cores."""
    G = len(spec)
    A_pad = sum(ag for ag, _ in spec)
    nRc = sum(nk for _, nk in spec)
    R_pad = nRc * P
    f32 = mybir.dt.float32
    bf16 = mybir.dt.bfloat16

    a_off = np.concatenate([[0], np.cumsum([ag for ag, _ in spec])])
    r_off = np.concatenate([[0], np.cumsum([nk * P for _, nk in spec])])
    k_off = np.concatenate([[0], np.cumsum([nk for _, nk in spec])])

    # packed input tensor column offsets (bf16 cols)
    MT0 = 0
    WV0 = P
    RT0 = 2 * P
    NA = RT0 + R_pad

    nc = bacc.Bacc("TRN2")
    inA = nc.dram_tensor("inA", [P, NA], bf16, kind="ExternalInput")
    atomT = nc.dram_tensor("atomT", [P, A_pad], bf16, kind="ExternalInput")
    uo = nc.dram_tensor("uo", [P, 2, A_pad], bf16, kind="ExternalOutput")

    from contextlib import ExitStack

    with tile.TileContext(nc) as tc:
        with (
            tc.tile_pool(name="singles", bufs=1) as singles,
            tc.tile_pool(name="ps_s", bufs=2, space="PSUM") as ps_s,
        ):
            kv_ctx = ExitStack()
            ps_kv = kv_ctx.enter_context(
                tc.tile_pool(name="ps_kv", bufs=2, space="PSUM")
            )
            inA_sb = singles.tile([P, NA], bf16)
            atomT_sb = singles.tile([P, A_pad], bf16)
            KMT_sb = singles.tile([P, R_pad], bf16)
            V_sb = singles.tile([P, R_pad], bf16)
            UO_sb = singles.tile([P, 2, A_pad], bf16)
            ES_sb = [
                singles.tile([P, nk, ag], bf16, name=f"es{j}")
                for j, (ag, nk) in enumerate(spec)
            ]
            warm_sb = singles.tile([P, 512], bf16)

            mT_v = inA_sb[:, MT0 : MT0 + P]
            wvT_v = inA_sb[:, WV0 : WV0 + P]
            resT_v = inA_sb[:, RT0 : RT0 + R_pad]

            # ---- input DMAs: [mw|resT0], atomT0, [resT rest], atomT x2 ----
            cut = RT0 + int(r_off[1])
            nc.sync.dma_start(inA_sb[:, :cut], inA[:, :cut])
            nc.sync.dma_start(atomT_sb[:, : int(a_off[1])],
                              atomT[:, : int(a_off[1])])
            nc.sync.dma_start(inA_sb[:, cut:], inA[:, cut:])
            a_cuts = [int(a_off[1]),
                      int(a_off[2]) if G > 2 else int(a_off[1]), A_pad]
            for lo, hi in zip(a_cuts[:-1], a_cuts[1:]):
                if hi > lo:
                    nc.sync.dma_start(atomT_sb[:, lo:hi], atomT[:, lo:hi])

            # ---- PE warm-up + ACT table preload during the DMA window ----
            scratch_sb = singles.tile([P, 1], bf16)
            nc.gpsimd.memset(warm_sb[:], 0.0)
            nc.scalar.activation(
                scratch_sb[:], warm_sb[:, :1],
                mybir.ActivationFunctionType.Exp, bias=0.0, scale=1.0,
            )
            for _ in range(NWARM):
                pw = ps_s.tile([P, 1024], f32, tag="s")
                nc.tensor.matmul(
                    pw[:, :512], warm_sb[:, :P], warm_sb[:], start=True, stop=True
                )

            # ---- KMT = M @ resT; chunks: [slot0][512-chunks of rest] ----
            kmt_cuts = [0, int(r_off[1])]
            while kmt_cuts[-1] < R_pad:
                kmt_cuts.append(min(kmt_cuts[-1] + 512, R_pad))
            for lo, hi in zip(kmt_cuts[:-1], kmt_cuts[1:]):
                pk = ps_kv.tile([P, 512], f32, tag="kv")
                nc.tensor.matmul(
                    pk[:, : hi - lo], mT_v, resT_v[:, lo:hi],
                    start=True, stop=True,
                )
                nc.vector.tensor_copy(KMT_sb[:, lo:hi], pk[:, : hi - lo])

            # ---- V = res @ Wv^T; 128-wide matmuls, 512-wide copies ----
            for lo in range(0, R_pad, 512):
                hi = min(lo + 512, R_pad)
                pv = ps_kv.tile([P, 512], f32, tag="kv")
                for c in range(lo, hi, P):
                    nc.tensor.matmul(
                        pv[:, c - lo : c - lo + P],
                        resT_v[:, c : c + P],
                        wvT_v,
                        start=True, stop=True,
                    )
                nc.vector.tensor_copy(V_sb[:, lo:hi], pv[:, : hi - lo])

            # KMT/V psum no longer needed; free its banks for ctx
            kv_ctx.close()
            c_ctx = ExitStack()
            ps_c = c_ctx.enter_context(
                tc.tile_pool(name="ps_c", bufs=2, space="PSUM")
            )

            # ---- slot pipeline ----
            def scores(j):
                ag, nk = spec[j]
                a0 = a_off[j]
                for k in range(nk):
                    kg = k_off[j] + k
                    r0 = r_off[j] + k * P
                    ps = ps_s.tile([P, 1024], f32, tag="s")
                    for c in range(0, ag, 512):
                        w = min(512, ag - c)
                        nc.tensor.matmul(
                            ps[:, c : c + w],
                            KMT_sb[:, r0 : r0 + P],
                            atomT_sb[:, a0 + c : a0 + c + w],
                            start=True, stop=True,
                        )
                    nc.scalar.activation(
                        ES_sb[j][:, k, :], ps[:, :ag],
                        mybir.ActivationFunctionType.Exp,
                        bias=0.0, scale=1.0,
                    )

            def partials(j, eng):
                ag, nk = spec[j]
                a0 = a_off[j]
                dst = UO_sb[:, 1, a0 : a0 + ag]
                if nk == 1:
                    eng.tensor_copy(dst, ES_sb[j][:, 0, :])
                else:
                    eng.tensor_add(dst, ES_sb[j][:, 0, :], ES_sb[j][:, 1, :])
                    for k in range(2, nk):
                        eng.tensor_add(dst, dst, ES_sb[j][:, k, :])

            def ctx(j):
                ag, nk = spec[j]
                a0 = a_off[j]
                pc = ps_c.tile([P, 1024], f32, tag="c")
                for c in range(0, ag, 512):
                    w = min(512, ag - c)
                    for k in range(nk):
                        r0 = r_off[j] + k * P
                        nc.tensor.matmul(
                            pc[:, c : c + w],
                            V_sb[:, r0 : r0 + P],
                            ES_sb[j][:, k, c : c + w],
                            start=(k == 0), stop=(k == nk - 1),
                        )
                nc.vector.tensor_copy(UO_sb[:, 0, a0 : a0 + ag], pc[:, :ag])

            def out_dma(j):
                ag, _ = spec[j]
                a0 = a_off[j]
                nc.sync.dma_start(
                    uo[:, :, a0 : a0 + ag], UO_sb[:, :, a0 : a0 + ag]
                )

            # partials: gpsimd is slow (~1.1us/add) but free — use it for
            # early slots whose out-DMA deadline is far; DVE (fast mode,
            # ~0.3us/add) for the last two slots on the tail.
            def peng(j):
                return nc.gpsimd if j < G - 2 else nc.vector

            scores(0)
            scores(1)
            partials(0, peng(0))
            ctx(0)
            out_dma(0)
            for j in range(2, G):
                scores(j)
                partials(j - 1, peng(j - 1))
                ctx(j - 1)
                out_dma(j - 1)
            partials(G - 1, nc.vector)
            # last slot: ship esum as soon as the partial lands, ctx after
            a0, ag = a_off[G - 1], spec[G - 1][0]
            nc.sync.dma_start(
                uo[:, 1, a0 : a0 + ag], UO_sb[:, 1, a0 : a0 + ag]
            )
            ctx(G - 1)
            nc.sync.dma_start(
                uo[:, 0, a0 : a0 + ag], UO_sb[:, 0, a0 : a0 + ag]
            )
            c_ctx.close()

    nc.compile()
    return nc


def _plan(ac, rc, G):
    """Assign graphs to (core, slot). Returns slot spec and assignment."""
    nkg = np.maximum(1, np.ceil(rc / P).astype(int))
    order = np.lexsort((-ac, -nkg))
    spec = []
    assign = []
    for j in range(G):
        grp = order[j * N_CORES : (j + 1) * N_CORES]
        nk = int(nkg[grp].max())
        ag = int(ac[grp].max())
        ag = max(64, (ag + 3) // 4 * 4)
        spec.append((ag, nk))
        assign.append(list(grp))
    return tuple(spec), assign


def kernel(atom_h, residue_h, atom_batch, residue_batch, W_q, W_k, W_v):
    atom_h = np.asarray(atom_h, dtype=np.float32)
    residue_h = np.asarray(residue_h, dtype=np.float32)
    atom_batch = np.asarray(atom_batch)
    residue_batch = np.asarray(residue_batch)
    W_q = np.asarray(W_q, dtype=np.float32)
    W_k = np.asarray(W_k, dtype=np.float32)
    W_v = np.asarray(W_v, dtype=np.float32)

    A = atom_h.shape[0]
    R = residue_h.shape[0]
    n_b = max(32, int(atom_batch.max()) + 1 if A else 1,
              int(residue_batch.max()) + 1 if R else 1)
    n_b = (n_b + N_CORES - 1) // N_CORES * N_CORES
    G = n_b // N_CORES

    ac = np.bincount(atom_batch, minlength=n_b)
    rc = np.bincount(residue_batch, minlength=n_b)
    a_seg = np.concatenate([[0], np.cumsum(ac)])
    r_seg = np.concatenate([[0], np.cumsum(rc)])

    spec, assign = _plan(ac, rc, G)
    a_off = np.concatenate([[0], np.cumsum([ag for ag, _ in spec])])
    k_off = np.concatenate([[0], np.cumsum([nk for _, nk in spec])])
    A_pad = int(a_off[-1])
    nRc = int(k_off[-1])
    R_pad = nRc * P
    RT0 = 2 * P
    NA = RT0 + R_pad

    if spec not in _kernel_cache:
        _kernel_cache[spec] = _build_kernel(spec)
    nc = _kernel_cache[spec]

    scale = 1.0 / np.sqrt(np.float32(DH))
    mT = np.ascontiguousarray((W_q.T @ W_k * scale).T).astype(BF16)
    wvT = np.ascontiguousarray(W_v.T).astype(BF16)

    atom_bf = atom_h.astype(BF16)
    res_bf = residue_h.astype(BF16)

    in_maps = []
    for c in range(N_CORES):
        inA_c = np.zeros((P, NA), dtype=BF16)
        inA_c[:, :P] = mT
        inA_c[:, P : 2 * P] = wvT
        atomT_c = np.zeros((P, A_pad), dtype=BF16)
        for j, (ag, nk) in enumerate(spec):
            g = assign[j][c]
            na, nr = int(ac[g]), int(rc[g])
            if na:
                atomT_c[:, a_off[j] : a_off[j] + na] = (
                    atom_bf[a_seg[g] : a_seg[g] + na].T
                )
            if nr:
                inA_c[:, RT0 + k_off[j] * P : RT0 + k_off[j] * P + nr] = (
                    res_bf[r_seg[g] : r_seg[g] + nr].T
                )
        in_maps.append({"inA": inA_c, "atomT": atomT_c})

    res = run_bass_kernel_spmd(nc, in_maps, core_ids=list(range(N_CORES)))

    result = atom_h.copy()
    for c in range(N_CORES):
        u = res.results[c]["uo"].astype(np.float32)
        for j, (ag, nk) in enumerate(spec):
            g = assign[j][c]
            na, nr = int(ac[g]), int(rc[g])
            if na == 0 or nr == 0:
                continue
            seg = slice(int(a_off[j]), int(a_off[j]) + na)
            # padded residues contribute exp(0)=1 each to the raw sum
            den = u[:, 1, seg].sum(axis=0) - np.float32(nk * P - nr)
            np.maximum(den, 1e-30, out=den)
            result[a_seg[g] : a_seg[g] + na] += (u[:, 0, seg] / den).T
    return result
